# revision 1
# baseline (speedup 1.0000x reference)
"""Trainium2 Bass kernel for nn_CBL_1632087573343 (boundary context loss).

Data-parallel over batch: 8 images -> 8 NeuronCores, one image per core.

Per-core pipeline (one image), final:
  - er is host-cast to bf16 and host-packed into half slabs
    [2 halves, 2 chunks, 128, 8512] (plus a 1-pixel-shifted xodd copy for
    4B-aligned odd-dx reads), so the device does plain contiguous HWDGE
    DMA loads (sync + scalar rings).
  - All label-derived quantities (per-shift label-similarity lab_s and
    fold weight W_s = valid + valid_s, the valid count, the include
    flag) are computed on the HOST from seg/gt_boundary and shipped as
    one bf16 plane tile; the device only computes the er-dependent part.
  - 12 shift product fields on DVE (bf16 2x tensor_tensor; odd-dx reads
    use the xodd slab); the norm field (er^2) on the ACT engine
    (activation Square).  GPSIMD is intentionally idle: its SBUF port
    contends with DVE 2x-mode and slows the products down.
  - PE channel-reduction via one-hot-column stationaries, c-major psum
    accumulation; ACT copies psum rows -> st (bf16), 4 sync-DMAs fan st
    out to dot field tiles [y=128, 192].
  - Pointwise per shift: cos = dot*rn*rn_s (DVE), d = cos - lab (DVE),
    e2 = d^2 (ACT), fw = e2*W (DVE), column-reduce into R (DVE);
    interleaved into the second half's shift loop so the tail is short.
  - Scheduling notes (measured on HW): the sync ring carries only the 8
    big slab loads (h1's issued late in h0 — issuing them upfront makes
    the DMA writes steal SBUF bandwidth from DVE and slows every product
    by ~20%); fanouts/labw/rn-shifts ride the scalar ring;
    tensor_tensor_reduce is avoided (fails at runtime on this HW).
Device returns S_i = sum_s sum_p W_s (cos_s - lab_s)^2; host computes
loss = sum_i [S_i / max(cnt_i,1) / 24 * include_i] / max(sum include, 1).
"""

import sys

sys.path.insert(0, "/opt/trn_rl_repo")

import numpy as np

import concourse.bass as bass
import concourse.tile as tile
from concourse import bacc, mybir

DT = mybir.dt
F32 = DT.float32
BF16 = DT.bfloat16
ALU = mybir.AluOpType
ACTF = mybir.ActivationFunctionType
AX = mybir.AxisListType

B, C, H, W = 8, 256, 128, 128
HH = 64                          # rows per half
SLAB_ROWS = HH + 2               # rows resident per half (dy<=2 read-ahead)
L_SLAB = 8512                    # >= 66*128+4, padded to a 128B multiple
L_RED = HH * W                   # 8192 columns reduced per (half, shift)
NB = 16                          # 512-pixel blocks per (half, shift)
FX = 192                         # field tile free size
FOFF = 2                         # x offset inside field tiles

# canonical half of the 24-shift set; even-dx first so odd-dx (xodd) use
# comes after the xo slab load
SHIFTS = [(1, 0), (2, 0), (0, 2), (1, -2), (1, 2), (2, -2), (2, 2),
          (0, 1), (1, -1), (1, 1), (2, -1), (2, 1)]
R_COL = {s: i for i, s in enumerate(SHIFTS)}
LFX = 24 * FX                    # host labw plane: 12 shifts x (lab, W)


def _ap(t, offset, dims):
    return bass.AP(t.tensor, offset, [list(d) for d in dims])


def build_kernel(nc):
    er_d = nc.dram_tensor("ers", [2, 2, 128, L_SLAB], BF16,
                          kind="ExternalInput")
    xo_d = nc.dram_tensor("xos", [2, 2, 128, L_SLAB], BF16,
                          kind="ExternalInput")
    lw_d = nc.dram_tensor("labw", [128, LFX], BF16, kind="ExternalInput")
    out_d = nc.dram_tensor("out", [1, 2], F32, kind="ExternalOutput")

    with tile.TileContext(nc) as tc:
        _build(tc, er_d, xo_d, lw_d, out_d)
    nc.compile()
    return nc


def _build(tc, er_d, xo_d, lw_d, out_d):
    nc = tc.nc
    from contextlib import ExitStack

    with ExitStack() as ctx:
        const_p = ctx.enter_context(tc.tile_pool(name="const", bufs=1))
        er_p = ctx.enter_context(tc.tile_pool(name="erp", bufs=2))
        xo_p = ctx.enter_context(tc.tile_pool(name="xop", bufs=1))
        prod_p = ctx.enter_context(tc.tile_pool(name="prodp", bufs=1))
        nprod_p = ctx.enter_context(tc.tile_pool(name="nprodp", bufs=1))
        field_p = ctx.enter_context(tc.tile_pool(name="fieldp", bufs=1))
        st_p = ctx.enter_context(tc.tile_pool(name="stp", bufs=3))
        scr_p = ctx.enter_context(tc.tile_pool(name="scrp", bufs=1))
        psum_p = ctx.enter_context(
            tc.tile_pool(name="psump", bufs=4, space="PSUM"))

        ones_f = const_p.tile([128, 32], F32, name="ones_f", tag="ones_f")
        nc.vector.memset(ones_f[:], 1.0)
        # one-hot column bank: sel[:, P0-b : P0+1] has its only nonzero
        # (ones) column at relative position b
        SELW = 320
        sel = const_p.tile([128, SELW], BF16, name="sel", tag="sel")
        nc.vector.memset(sel[:], 0.0)
        nc.vector.memset(sel[:, 128 + NB - 1:128 + NB], 1.0)
        P0 = 128 + NB - 1

        def sel_view(b):
            return sel[:, P0 - b:P0 + 1]

        # ---- host-computed label/weight planes (DMA issued late so the
        # er slab loads win the SDMA bandwidth race) ---------------------
        labw = const_p.tile([128, LFX], BF16, name="labw", tag="labw")

        def lab_view(s):
            o = 2 * R_COL[s] * FX
            return labw[:, o + FOFF:o + FOFF + W]

        def w_view(s):
            o = (2 * R_COL[s] + 1) * FX
            return labw[:, o + FOFF:o + FOFF + W]

        R = scr_p.tile([128, 32], F32, name="R", tag="R")
        nc.vector.memset(R[:], 0.0)

        # ---- dot fields ([y, x]); norm field f32, shifts bf16 ----------
        n2f = field_p.tile([H, FX], F32, name="n2f", tag="n2f")
        nc.vector.memset(n2f[:], 0.0)
        fields = {}
        for s in SHIFTS:
            f = field_p.tile([H, FX], BF16, name=f"dot_{s[0]}_{s[1]}",
                             tag=f"dot_{s[0]}_{s[1]}")
            nc.vector.memset(f[:], 0.0)
            fields[s] = f

        # ---- per-(half, shift) PE reduction + fanout helper ------------
        def reduce_and_fanout(prods, s, h, is_norm):
            r0 = HH * h
            ps = psum_p.tile([128, 512], F32, name="ps", tag="ps")
            n_mm = 2 * NB
            j = 0
            # c-major: chunk 0's 16 blocks, then chunk 1 accumulates.
            for c in range(2):
                for b in reversed(range(NB)):
                    nc.tensor.matmul(
                        ps[0:b + 1, 0:512], sel_view(b),
                        _ap(prods[c], 128 * b,
                            [[L_RED, 128], [128 * NB, 4], [1, W]]),
                        start=(j == 0), stop=(j == n_mm - 1),
                        skip_group_check=True)
                    j += 1

            if is_norm:
                st = st_p.tile([NB, 512], F32, name="stf", tag="stf")
                f = n2f
            else:
                st = st_p.tile([NB, 512], BF16, name="stb", tag="stb")
                f = fields[s]
            nc.scalar.copy(st[:], ps[0:NB, 0:512])
            # fanout on the scalar ring: the sync ring carries only the
            # big slab loads so they are never queued behind fanouts
            for q in range(4):
                nc.scalar.dma_start(
                    out=_ap(f, (r0 + 16 * q) * FX + FOFF,
                            [[FX, NB], [1, W]]),
                    in_=_ap(st, 128 * q, [[512, NB], [1, W]]))

        # ---- pointwise helpers -----------------------------------------
        b_ = np.s_[:, FOFF:FOFF + W]
        rshift = {}

        def rn_chain():
            # rn = 1 / max(sqrt(n2), eps); bf16 copy + dy-shifted copies
            rn1 = scr_p.tile([H, FX], F32, name="rn1", tag="rn1")
            nc.scalar.sqrt(rn1[:], n2f[:])
            nc.vector.tensor_scalar(rn1[:], rn1[:], 1e-8, None,
                                    op0=ALU.max)
            rnf = scr_p.tile([H, FX], F32, name="rnf", tag="rnf")
            nc.vector.reciprocal(rnf[:], rn1[:])
            rn = field_p.tile([H, FX], BF16, name="rn", tag="rn")
            nc.vector.tensor_copy(rn[:], rnf[:])
            rshift[0] = rn
            for k in (1, 2):
                t = field_p.tile([H, FX], BF16, name=f"rn_d{k}",
                                 tag=f"rn_d{k}")
                nc.vector.memset(t[:], 0)
                nc.scalar.dma_start(
                    out=_ap(t, 0, [[FX, H - k], [1, FX]]),
                    in_=_ap(rn, k * FX, [[FX, H - k], [1, FX]]))
                rshift[k] = t

        def pointwise(s):
            dy, dx = s
            sh = np.s_[:, FOFF + dx:FOFF + dx + W]
            rn = rshift[0]
            rn_s = rshift[dy]
            t1 = scr_p.tile([H, FX], BF16, name="t1", tag="t1")
            nc.vector.tensor_tensor(t1[b_], fields[s][b_], rn[b_],
                                    op=ALU.mult)
            cosb = scr_p.tile([H, FX], BF16, name="cosb", tag="cosb")
            nc.vector.tensor_tensor(cosb[b_], t1[b_], rn_s[sh],
                                    op=ALU.mult)
            d = scr_p.tile([H, FX], BF16, name="d", tag="d")
            nc.vector.tensor_tensor(d[b_], cosb[b_], lab_view(s),
                                    op=ALU.subtract)
            e2 = scr_p.tile([H, FX], BF16, name="e2", tag="e2")
            nc.scalar.square(e2[b_], d[b_])
            fw = scr_p.tile([H, FX], BF16, name="fw", tag="fw")
            nc.vector.tensor_tensor(fw[b_], e2[b_], w_view(s),
                                    op=ALU.mult)
            col = R_COL[s]
            nc.vector.tensor_reduce(R[:, col:col + 1], fw[b_], axis=AX.X,
                                    op=ALU.add)

        # ---- slab loads, both halves up front --------------------------
        # sync ring carries ONLY the 8 big loads, in need-order: er h0,
        # xo h0, er h1, xo h1 (the xo h1 writes WAR-wait on h0's odd
        # products, but nothing queues behind them).  labw + fanouts +
        # small copies ride the scalar ring.
        er_h, xo_h = [], []

        def emit_loads(h):
            er_ch = []
            for c in range(2):
                e = er_p.tile([128, L_SLAB], BF16, name=f"er{c}",
                              tag=f"er{c}")
                nc.sync.dma_start(
                    out=e[:],
                    in_=_ap(er_d.ap(), (h * 2 + c) * 128 * L_SLAB,
                            [[L_SLAB, 128], [1, L_SLAB]]))
                er_ch.append(e)
            er_h.append(er_ch)
            xo_ch = []
            for c in range(2):
                x = xo_p.tile([128, L_SLAB], BF16, name=f"xo{c}",
                              tag=f"xo{c}")
                nc.sync.dma_start(
                    out=x[:],
                    in_=_ap(xo_d.ap(), (h * 2 + c) * 128 * L_SLAB,
                            [[L_SLAB, 128], [1, L_SLAB]]))
                xo_ch.append(x)
            xo_h.append(xo_ch)

        emit_loads(0)

        def alloc_norm_tiles():
            return [nprod_p.tile([128, L_RED], BF16, name=f"np{c}",
                                 tag=f"np{c}") for c in range(2)]

        def emit_norm_half(tiles, h, q):
            # halved Square ops so interleaved st copies are not queued
            # behind 14us of contiguous ACT work (psum backpressure)
            for c in range(2):
                nc.scalar.square(tiles[c][:, 4096 * q:4096 * (q + 1)],
                                 er_h[h][c][:, 4096 * q:4096 * (q + 1)])

        # ---- main per-half loop ----------------------------------------
        nprods = {}
        for h in range(2):
            er_ch = er_h[h]
            nprods[h] = alloc_norm_tiles()
            emit_norm_half(nprods[h], h, 0)

            for i, s in enumerate(SHIFTS):
                dy, dx = s
                off = dy * W + dx
                prods = []
                for c in range(2):
                    p = prod_p.tile([128, L_RED], BF16, name=f"p{c}",
                                    tag=f"prod{c}")
                    if dx % 2 == 0:
                        in1 = er_ch[c][:, off:off + L_RED]
                    else:
                        in1 = xo_h[h][c][:, off - 1:off - 1 + L_RED]
                    nc.vector.tensor_tensor(
                        p[:], er_ch[c][:, 0:L_RED], in1, op=ALU.mult)
                    prods.append(p)
                reduce_and_fanout(prods, s, h, False)
                if h == 0:
                    if i == 0:
                        # second norm halves + labw load issued after the
                        # first st copy is queued on the ACT ring
                        emit_norm_half(nprods[0], 0, 1)
                        nc.scalar.dma_start(out=labw[:], in_=lw_d.ap())
                    if i == 1:
                        reduce_and_fanout(nprods[0], (0, 0), 0, True)
                    if i == 10:
                        # h1 slab loads: late enough that the DMA writes
                        # don't steal SBUF bandwidth from h0's products,
                        # early enough to land by the h0/h1 boundary
                        emit_loads(1)
                else:
                    if i == 0:
                        emit_norm_half(nprods[1], 1, 1)
                        reduce_and_fanout(nprods[1], (0, 0), 1, True)
                    if i == 2:
                        rn_chain()
                    if i >= 3:
                        pointwise(SHIFTS[i - 3])
            if h == 1:
                for k in range(9, 12):
                    pointwise(SHIFTS[k])

        # ---- final reduction: S = sum over R columns & partitions ------
        ps2 = psum_p.tile([128, 512], F32, name="ps2", tag="ps")
        nc.tensor.matmul(ps2[0:1, 0:12], ones_f[:, 0:1], R[:, 0:12],
                         start=True, stop=True)
        scal = scr_p.tile([1, 32], F32, name="scal", tag="scal")
        nc.scalar.copy(scal[0:1, 0:12], ps2[0:1, 0:12])
        nc.vector.tensor_reduce(scal[0:1, 16:17], scal[0:1, 0:12],
                                axis=AX.X, op=ALU.add)

        outt = scr_p.tile([1, 32], F32, name="outt", tag="outt")
        nc.vector.tensor_copy(outt[0:1, 0:1], scal[0:1, 16:17])
        nc.vector.memset(outt[0:1, 1:2], 0.0)
        nc.sync.dma_start(out=out_d.ap(), in_=outt[0:1, 0:2])


_NC_CACHE = {}


def get_nc():
    if "nc" not in _NC_CACHE:
        nc = bacc.Bacc("TRN2", target_bir_lowering=False, debug=False)
        build_kernel(nc)
        _NC_CACHE["nc"] = nc
    return _NC_CACHE["nc"]


def _prep_slabs(er):
    """er f32 [B, C, H, W] -> (er_slabs, xo_slabs) bf16
    [B, 2 halves, 2 chunks, 128, L_SLAB]."""
    import ml_dtypes

    erb = np.ascontiguousarray(er.reshape(B, 2, 128, H * W)).astype(
        ml_dtypes.bfloat16)
    ers = np.zeros((B, 2, 2, 128, L_SLAB), dtype=ml_dtypes.bfloat16)
    xos = np.zeros((B, 2, 2, 128, L_SLAB), dtype=ml_dtypes.bfloat16)
    n0 = SLAB_ROWS * W                       # 8448 (h=0)
    n1 = HH * W                              # 8192 (h=1)
    ers[:, 0, :, :, :n0] = erb[:, :, :, 0:n0]
    ers[:, 1, :, :, :n1] = erb[:, :, :, n1:2 * n1]
    xos[:, 0, :, :, :n0] = erb[:, :, :, 1:n0 + 1]
    xos[:, 1, :, :, :n1 - 1] = erb[:, :, :, n1 + 1:2 * n1]
    return ers, xos


def _prep_labels(seg, gtb):
    """Host label prep: per-image labw plane [128, LFX] bf16 plus
    (cnt, include) per image."""
    import ml_dtypes

    seg0 = np.where(seg == 255, 0, seg)
    gtb0 = np.where(gtb == 255, 0, gtb)
    gt_b = (gtb0 * seg0).astype(np.int64)            # [B, H, W]
    interior = np.zeros((H, W), bool)
    interior[2:H - 2, 2:W - 2] = True
    valid = (gt_b > 0) & interior                    # [B, H, W]
    include = (gt_b > 0).any(axis=(1, 2)).astype(np.float64)
    cnt = valid.sum(axis=(1, 2)).astype(np.float64)

    labw = np.zeros((B, 128, LFX), dtype=ml_dtypes.bfloat16)
    vf = valid.astype(np.float32)
    for s_i, (dy, dx) in enumerate(SHIFTS):
        seg_s = np.roll(seg, (-dy, -dx), axis=(1, 2))
        lab = ((seg == seg_s) & (seg < 2)).astype(np.float32)
        v_s = np.zeros_like(vf)
        v_s[:, :H - dy, :] = vf[:, dy:, :]
        w = np.zeros_like(vf)
        if dx >= 0:
            w[:, :, :W - dx] = v_s[:, :, dx:]
        else:
            w[:, :, -dx:] = v_s[:, :, :W + dx]
        w += vf
        labw[:, :, 2 * s_i * FX + FOFF:2 * s_i * FX + FOFF + W] = lab
        labw[:, :, (2 * s_i + 1) * FX + FOFF:(2 * s_i + 1) * FX + FOFF + W] = w
    return labw, cnt, include


def kernel(er_input, seg_label, gt_boundary_seg):
    er = np.ascontiguousarray(np.asarray(er_input, dtype=np.float32))
    seg = np.ascontiguousarray(np.asarray(seg_label, dtype=np.int32))
    gtb = np.ascontiguousarray(np.asarray(gt_boundary_seg, dtype=np.int32))
    assert er.shape == (B, C, H, W), er.shape

    ers, xos = _prep_slabs(er)
    labw, cnt, include = _prep_labels(seg, gtb)
    nc = get_nc()
    from concourse.bass_utils import run_bass_kernel_spmd

    in_maps = [
        {"ers": ers[i], "xos": xos[i], "labw": labw[i]} for i in range(B)
    ]
    res = run_bass_kernel_spmd(nc, in_maps, list(range(B)))
    S = np.array([res.results[i]["out"][0, 0] for i in range(B)],
                 dtype=np.float64)
    loss_i = S / np.maximum(cnt, 1.0) / 24.0 * include
    loss = loss_i.sum() / max(include.sum(), 1.0)
    return np.float32(loss)



# revision 2
# speedup vs baseline: 1.9964x; 1.9964x over previous
"""Trainium2 Bass kernel for nn_CBL_1632087573343 (boundary context loss).

Data-parallel over batch: 8 images -> 8 NeuronCores, one image per core.

Per-core pipeline (one image), Gram-matrix formulation:
  - er is host-cast to bf16 and packed into 8 range-slabs
    [2 chunks, 4 row-ranges, 128, RSLAB] so each 32-row band (plus 2
    rows of read-ahead) is an independent SBUF tile, letting the PE
    start before the full image has loaded.
  - For every row y and 32-pixel group g the PE computes a narrow Gram
    block G[m, 36r + w] = dot_c(er[:, y, 32g+m], er[:, y+r, 32g-2+w])
    (contraction over the 128-channel chunks, accumulated in PSUM,
    tile_position=(0, 32g) stacks the 4 groups in PE array columns).
    Every cosine numerator AND the squared-norm field are diagonals of
    these blocks -- no elementwise product pass and no one-hot
    reduction is needed at all.
  - Diagonals cannot be extracted on-chip (engine reads are
    partition-uniform; SBUF DMA partition-step drift wraps mod 16 B),
    so the blocks bounce through a DRAM scratch: PSUM->SBUF copy
    (ACT/DVE alternating), contiguous write at row pitch SM=112, and a
    stride-113 readback that turns the shear into a legal strided DMA.
    One readback per (group, 32-row range) lands all 13 dot-fields in
    [x, 80*y + c] layout (c = 36*dy + dx + 2).
  - Labels (lab) and fold weights (W = valid + valid_s) are
    host-computed in transposed [x, y] layout; the device only does the
    tiny pointwise phase: cos = dot*rn*rn_s, (cos-lab)^2 * W, reduce.
Device returns S_i = sum_s sum_p W_s (cos_s - lab_s)^2; host computes
loss = sum_i [S_i / max(cnt_i,1) / 24 * include_i] / max(sum include, 1).
"""

import sys

sys.path.insert(0, "/opt/trn_rl_repo")

import numpy as np

import concourse.bass as bass
import concourse.tile as tile
from concourse import bacc, mybir

DT = mybir.dt
F32 = DT.float32
BF16 = DT.bfloat16
ALU = mybir.AluOpType
AX = mybir.AxisListType

B, C, H, W = 8, 256, 128, 128
NR = 4                           # row ranges (32 rows each + 2 readahead)
RROWS = 34                       # rows resident per range tile
RSLAB = 2 + RROWS * W + 130      # 4484: front pad 2, back pad
SM = 112                         # DRAM scratch row pitch (per pixel m)
SG = 32 * SM                     # 3584: per (y, group) block
SY = 4 * SG                      # 14336: per y
RB = 77                          # readback window: c = 36*dy + dx + 2
FYP = 80                         # fld per-y pitch (>= RB)
FP = FYP * H                     # 10240: fld cols per partition
LH = 128                         # labw per-plane pitch

# canonical half of the 24-shift set (mirror folded into W on host)
SHIFTS = [(0, 1), (0, 2), (1, -2), (1, -1), (1, 0), (1, 1), (1, 2),
          (2, -2), (2, -1), (2, 0), (2, 1), (2, 2)]


def _ap(t, offset, dims):
    return bass.AP(t.tensor, offset, [list(d) for d in dims])


def build_kernel(nc):
    er_d = nc.dram_tensor("ers", [2, NR, 128, RSLAB], BF16,
                          kind="ExternalInput")
    lw_d = nc.dram_tensor("labw", [128, 24 * LH], BF16,
                          kind="ExternalInput")
    out_d = nc.dram_tensor("out", [1, 2], F32, kind="ExternalOutput")

    with tile.TileContext(nc) as tc:
        _build(tc, er_d, lw_d, out_d)
    nc.compile()
    return nc


def _build(tc, er_d, lw_d, out_d):
    nc = tc.nc
    from contextlib import ExitStack

    with ExitStack() as ctx:
        const_p = ctx.enter_context(tc.tile_pool(name="const", bufs=1))
        er_p = ctx.enter_context(tc.tile_pool(name="erp", bufs=1))
        g4_p = ctx.enter_context(tc.tile_pool(name="g4p", bufs=3))
        fld_p = ctx.enter_context(tc.tile_pool(name="fldp", bufs=1))
        scr_p = ctx.enter_context(tc.tile_pool(name="scrp", bufs=1))
        psum_p = ctx.enter_context(
            tc.tile_pool(name="psump", bufs=4, space="PSUM"))
        dram_p = ctx.enter_context(
            tc.tile_pool(name="dramp", bufs=1, space="DRAM"))

        ones_f = const_p.tile([128, 16], F32, name="ones_f", tag="ones_f")
        nc.vector.memset(ones_f[:], 1.0)
        R = const_p.tile([128, 16], F32, name="R", tag="R")
        nc.vector.memset(R[:], 0.0)

        labw = const_p.tile([128, 24 * LH], BF16, name="labw", tag="labw")
        nc.scalar.dma_start(out=labw[:], in_=lw_d.ap())

        # ---- er range-slab loads (sync ring, range-major) --------------
        er = [[None] * NR for _ in range(2)]
        for r in range(NR):
            for c in range(2):
                e = er_p.tile([128, RSLAB], BF16, name=f"er{c}_{r}",
                              tag=f"er{c}_{r}")
                nc.sync.dma_start(
                    out=e[:],
                    in_=_ap(er_d.ap(), (c * NR + r) * 128 * RSLAB,
                            [[RSLAB, 128], [1, RSLAB]]))
                er[c][r] = e

        fld = fld_p.tile([128, FP], BF16, name="fld", tag="fld")
        scratch = [dram_p.tile([1, 32 * SY], BF16, name=f"scr{r}",
                               tag=f"scr{r}") for r in range(NR)]

        # ---- main loop: 32 blocks of 4 rows ----------------------------
        for yb in range(32):
            ri = yb // 8
            ps = psum_p.tile([128, 512], F32, name="ps", tag="ps")
            for q in range(4):
                y = 4 * yb + q
                ry = y - 32 * ri
                nrows = min(3, 128 - y)
                for g in range(4):
                    for c in range(2):
                        base = 2 + ry * W + 32 * g
                        st = er[c][ri][:, base:base + 32]
                        mov = _ap(er[c][ri], base - 2,
                                  [[RSLAB, 128], [W, nrows], [1, 36]])
                        nc.tensor.matmul(
                            ps[32 * g:32 * g + 32,
                               108 * q:108 * q + 36 * nrows],
                            st, mov, start=(c == 0), stop=(c == 1),
                            skip_group_check=True,
                            tile_position=(0, 32 * g))
            g4 = g4_p.tile([128, 432], BF16, name="g4", tag="g4")
            if yb % 2 == 0:
                nc.scalar.copy(g4[:], ps[0:128, 0:432])
            else:
                nc.vector.tensor_copy(g4[:], ps[0:128, 0:432])
            # scratch write: addr = y_local*SY + p*SM + col
            nc.scalar.dma_start(
                out=_ap(scratch[ri], (yb % 8) * 4 * SY,
                        [[SM, 128], [SY, 4], [1, 108]]),
                in_=_ap(g4, 0, [[432, 128], [108, 4], [1, 108]]))
            if yb % 8 == 7:
                # shear readback: (m, y_local, c) from
                # y_local*SY + g*SG + 113*m + c -> fld[32g+m, 80*y + c]
                for g in range(4):
                    nc.sync.dma_start(
                        out=_ap(fld, 32 * g * FP + FYP * 32 * ri,
                                [[FP, 32], [FYP, 32], [1, RB]]),
                        in_=_ap(scratch[ri], g * SG,
                                [[113, 32], [SY, 32], [1, RB]]))

        # ---- rn chain: rn = 1/max(sqrt(norm2), eps) --------------------
        rn1 = scr_p.tile([128, 128], F32, name="rn1", tag="rn1")
        nc.scalar.sqrt(rn1[:], _ap(fld, 2, [[FP, 128], [FYP, 128]]))
        nc.vector.tensor_scalar(rn1[:], rn1[:], 1e-8, None, op0=ALU.max)
        rnf = scr_p.tile([128, 128], F32, name="rnf", tag="rnf")
        nc.vector.reciprocal(rnf[:], rn1[:])
        rn = scr_p.tile([128, 132], BF16, name="rn", tag="rn")
        nc.vector.memset(rn[:], 0.0)
        nc.vector.tensor_copy(rn[:, 0:128], rnf[:])
        rshift = {0: rn}
        for dx in (-2, -1, 1, 2):
            t = scr_p.tile([128, 132], BF16, name=f"rn_d{dx}",
                           tag=f"rn_d{dx}")
            nc.vector.memset(t[:], 0.0)
            if dx > 0:
                nc.scalar.dma_start(out=t[0:128 - dx, :],
                                    in_=rn[dx:128, :])
            else:
                nc.scalar.dma_start(out=t[-dx:128, :],
                                    in_=rn[0:128 + dx, :])
            rshift[dx] = t

        # ---- pointwise per canonical shift -----------------------------
        for i, (dy, dx) in enumerate(SHIFTS):
            c_idx = 36 * dy + dx + 2
            fldp = _ap(fld, c_idx, [[FP, 128], [FYP, 128]])
            t1 = scr_p.tile([128, 128], BF16, name=f"t1_{i}", tag="t1")
            nc.vector.tensor_tensor(t1[:], fldp, rn[:, 0:128],
                                    op=ALU.mult)
            rs = rshift[dx]
            cosb = scr_p.tile([128, 128], BF16, name=f"cos_{i}",
                              tag="cosb")
            nc.vector.tensor_tensor(cosb[:], t1[:], rs[:, dy:dy + 128],
                                    op=ALU.mult)
            d = scr_p.tile([128, 128], BF16, name=f"d_{i}", tag="d")
            nc.vector.tensor_tensor(d[:], cosb[:],
                                    labw[:, 2 * i * LH:2 * i * LH + 128],
                                    op=ALU.subtract)
            e2 = scr_p.tile([128, 128], BF16, name=f"e2_{i}", tag="e2")
            nc.scalar.square(e2[:], d[:])
            fw = scr_p.tile([128, 128], BF16, name=f"fw_{i}", tag="fw")
            nc.vector.tensor_tensor(
                fw[:], e2[:],
                labw[:, (2 * i + 1) * LH:(2 * i + 1) * LH + 128],
                op=ALU.mult)
            nc.vector.tensor_reduce(R[:, i:i + 1], fw[:], axis=AX.X,
                                    op=ALU.add)

        # ---- final reduction: S = sum over R columns & partitions ------
        ps2 = psum_p.tile([128, 512], F32, name="ps2", tag="ps")
        nc.tensor.matmul(ps2[0:1, 0:12], ones_f[:, 0:1], R[:, 0:12],
                         start=True, stop=True)
        scal = scr_p.tile([1, 32], F32, name="scal", tag="scal")
        nc.scalar.copy(scal[0:1, 0:12], ps2[0:1, 0:12])
        nc.vector.tensor_reduce(scal[0:1, 16:17], scal[0:1, 0:12],
                                axis=AX.X, op=ALU.add)

        outt = scr_p.tile([1, 32], F32, name="outt", tag="outt")
        nc.vector.tensor_copy(outt[0:1, 0:1], scal[0:1, 16:17])
        nc.vector.memset(outt[0:1, 1:2], 0.0)
        nc.sync.dma_start(out=out_d.ap(), in_=outt[0:1, 0:2])


_NC_CACHE = {}


def get_nc():
    if "nc" not in _NC_CACHE:
        nc = bacc.Bacc("TRN2", target_bir_lowering=False, debug=False)
        build_kernel(nc)
        _NC_CACHE["nc"] = nc
    return _NC_CACHE["nc"]


def _prep_slabs(er):
    """er f32 [B, C, H, W] -> bf16 range slabs [B, 2, NR, 128, RSLAB]."""
    import ml_dtypes

    erb = np.ascontiguousarray(er.reshape(B, 2, 128, H * W)).astype(
        ml_dtypes.bfloat16)
    ers = np.zeros((B, 2, NR, 128, RSLAB), dtype=ml_dtypes.bfloat16)
    for r in range(NR):
        lo = 32 * r * W
        hi = min((32 * r + RROWS) * W, H * W)
        ers[:, :, r, :, 2:2 + hi - lo] = erb[:, :, :, lo:hi]
    return ers


def _prep_labels(seg, gtb):
    """Host label prep in transposed [x, y] layout: labw [B, 128, 24*LH]
    bf16 plus (cnt, include) per image."""
    import ml_dtypes

    seg0 = np.where(seg == 255, 0, seg)
    gtb0 = np.where(gtb == 255, 0, gtb)
    gt_b = (gtb0 * seg0).astype(np.int64)            # [B, H, W]
    interior = np.zeros((H, W), bool)
    interior[2:H - 2, 2:W - 2] = True
    valid = (gt_b > 0) & interior                    # [B, H, W]
    include = (gt_b > 0).any(axis=(1, 2)).astype(np.float64)
    cnt = valid.sum(axis=(1, 2)).astype(np.float64)

    labw = np.zeros((B, 128, 24 * LH), dtype=ml_dtypes.bfloat16)
    vf = valid.astype(np.float32)
    for s_i, (dy, dx) in enumerate(SHIFTS):
        seg_s = np.roll(seg, (-dy, -dx), axis=(1, 2))
        lab = ((seg == seg_s) & (seg < 2)).astype(np.float32)
        v_s = np.zeros_like(vf)
        v_s[:, :H - dy, :] = vf[:, dy:, :]
        w = np.zeros_like(vf)
        if dx >= 0:
            w[:, :, :W - dx] = v_s[:, :, dx:]
        else:
            w[:, :, -dx:] = v_s[:, :, :W + dx]
        w += vf
        # transpose [y, x] -> [x, y]
        labw[:, :, 2 * s_i * LH:2 * s_i * LH + 128] = \
            lab.transpose(0, 2, 1)
        labw[:, :, (2 * s_i + 1) * LH:(2 * s_i + 1) * LH + 128] = \
            w.transpose(0, 2, 1)
    return labw, cnt, include


def kernel(er_input, seg_label, gt_boundary_seg):
    er = np.ascontiguousarray(np.asarray(er_input, dtype=np.float32))
    seg = np.ascontiguousarray(np.asarray(seg_label, dtype=np.int32))
    gtb = np.ascontiguousarray(np.asarray(gt_boundary_seg, dtype=np.int32))
    assert er.shape == (B, C, H, W), er.shape

    ers = _prep_slabs(er)
    labw, cnt, include = _prep_labels(seg, gtb)
    nc = get_nc()
    from concourse.bass_utils import run_bass_kernel_spmd

    in_maps = [
        {"ers": ers[i], "labw": labw[i]} for i in range(B)
    ]
    res = run_bass_kernel_spmd(nc, in_maps, list(range(B)))
    S = np.array([res.results[i]["out"][0, 0] for i in range(B)],
                 dtype=np.float64)
    loss_i = S / np.maximum(cnt, 1.0) / 24.0 * include
    loss = loss_i.sum() / max(include.sum(), 1.0)
    return np.float32(loss)
